# revision 38
# baseline (speedup 1.0000x reference)
"""GNN message-passing kernel for 8 TRN2 NeuronCores (Bass/Tile, SPMD).

Takes the FULL inputs of nn_Base_40793599378196 and returns the FULL
[512, 130] output. Internally:

- Nodes/graphs are sharded by graph: core c owns nodes [c*8192, (c+1)*8192)
  (64 graphs of 128 nodes). All weights are replicated.
- Per layer: y = h @ wn computed locally (node-major, bf16), AllGathered in
  two pipelined halves into a per-core replica of the full [65536, 128]
  message table (y_full[:HALF] = every core's nodes 0:4096, y_full[HALF:]
  the rest, so each gather half waits only on its own collective); edges
  (sharded by dst core, grouped into 128-node dst windows, split by src
  half for int16 indices) are gathered with the gpsimd dma_gather custom op
  on rotating SWDGE queues (a single queue throttles descriptor generation
  on ring back-pressure) and scatter-added via one-hot matmuls accumulating
  in PSUM; BatchNorm partial sums are computed per window as the scatter
  finishes (overlapping the gathers) and AllReduced; bias folds into the BN
  shift.
- Activations stay feature-major [128, 8192] in SBUF the whole time.
- Graph pooling = free-dim window reduction; graph head + 128 per-node head
  MLPs run on the 64 local graphs (batch-dim sharding).

Compute dtype: bf16 operands with fp32 PSUM/statistics (rel err vs the fp32
reference ~8e-3).
"""

import os

import numpy as np
import ml_dtypes

import concourse.bacc as bacc
import concourse.tile as tile
import concourse.mybir as mybir
from concourse.bass_utils import run_bass_kernel_spmd

F32 = mybir.dt.float32
BF16 = mybir.dt.bfloat16
I16 = mybir.dt.int16
AF = mybir.ActivationFunctionType
OP = mybir.AluOpType

NBF = ml_dtypes.bfloat16

N = 65536
E = 524288
H = 128
B = 512
NPG = 128
NC = 8
NPC = N // NC      # 8192 nodes per core
W = 128            # dst window (psum tile) width in nodes
NW = NPC // W      # 64 windows per core
WPB = 8            # windows per gather batch
NB = NW // WPB     # 8 batches per core
HALF = N // 2
GPC = B // NC      # 64 graphs per core
EPS = 1e-5


# ----------------------------------------------------------------- host prep

def _build_edge_plan(edge_index):
    src = edge_index[0].astype(np.int64)
    dst = edge_index[1].astype(np.int64)
    core = dst // NPC
    win = (dst % NPC) // W
    dst_rel = (dst % W).astype(np.float32)
    # The message table is AllGathered in two stages: y_full[:HALF] holds
    # every core's first NPC/2 nodes ([core][0:4096] blocks), y_full[HALF:]
    # the rest. An edge is "hi" if its src is in the second half of its
    # owning core, and table rows are core*4096 + (m % 4096).
    sc = src // NPC
    m = src % NPC
    is_hi = m >= (NPC // 2)
    row = sc * (NPC // 2) + (m % (NPC // 2))

    key = ((core * NW + win) * 2 + is_hi).astype(np.int64)
    order = np.argsort(key, kind="stable")
    key_s = key[order]
    src_s = row[order]
    dst_rel_s = dst_rel[order]

    counts = np.bincount(key_s, minlength=NC * NW * 2).reshape(NC, NW, 2)
    starts = np.zeros(NC * NW * 2 + 1, dtype=np.int64)
    np.cumsum(counts.reshape(-1), out=starts[1:])

    # Exact packing: within each (batch, half) gather segment, window w gets
    # a slot range of max-over-cores width at an arbitrary (unaligned)
    # offset; a 128-slot chunk can span two adjacent windows, which the
    # scatter disambiguates with dstrel + 128*(wi%2) and a shifted iota.
    maxcnt = counts.max(axis=0)  # [NW, 2]
    off = np.zeros((NW, 2), np.int64)
    span = np.zeros((NW, 2, 2), np.int64)  # chunk [c0, c1] within segment
    seg_nch = np.zeros((NB, 2), np.int64)
    for b in range(NB):
        for h in range(2):
            o = 0
            for wi in range(WPB):
                w = b * WPB + wi
                off[w, h] = o
                span[w, h, 0] = o // 128
                span[w, h, 1] = (o + maxcnt[w, h] - 1) // 128
                o += int(maxcnt[w, h])
            seg_nch[b, h] = (o + 127) // 128
            assert span[:, h, 1].max() - span[:, h, 0].min() >= 0
    seg_base = np.zeros((NB, 2), np.int64)  # slot base of each segment
    o = 0
    for b in range(NB):
        for h in range(2):
            seg_base[b, h] = o
            o += int(seg_nch[b, h]) * 128
    nslots = int(o)

    # Padding slots still issue real fetches; spread them over the table
    # instead of hammering row 0 (a single-row hot spot serializes on one
    # HBM bank queue and measurably slows the whole drain).
    spread = ((np.arange(nslots, dtype=np.int64) * 9973) % HALF).astype(np.int16)
    idx_all = np.tile(spread, (NC, 1))
    dstrel_all = np.full((NC, nslots), 511.0, dtype=np.float32)

    for c in range(NC):
        for b in range(NB):
            for h in range(2):
                base = seg_base[b, h]
                for wi in range(WPB):
                    w = b * WPB + wi
                    k = (c * NW + w) * 2 + h
                    s0, s1 = starts[k], starts[k + 1]
                    n = s1 - s0
                    o = base + off[w, h]
                    # sort the window's edges by source row: each SDMA
                    # engine then walks monotonically increasing HBM
                    # addresses, improving bank locality of the drain
                    e_src = src_s[s0:s1]
                    o2 = np.argsort(e_src, kind="stable")
                    idx_all[c, o : o + n] = e_src[o2].astype(np.int16)
                    dstrel_all[c, o : o + n] = dst_rel_s[s0:s1][o2] + 128 * (
                        wi % 2
                    )

    plan = dict(
        seg_nch=seg_nch,
        seg_base=seg_base,
        span=span,
        nslots=nslots,
        maxspan=int((span[:, :, 1] - span[:, :, 0] + 1).max()),
    )
    return plan, idx_all, dstrel_all


def _wrap_idx16(idx_flat, seg):
    """Wrap each `seg`-sized segment into dma_gather layout and concatenate:
    [S] -> [128, S/16] (index i of a segment at partition i%16, col i//16,
    replicated over the 8 groups of 16 partitions)."""
    s = idx_flat.shape[0]
    assert s % seg == 0 and seg % 16 == 0
    blocks = []
    for o in range(0, s, seg):
        a = idx_flat[o : o + seg].reshape(seg // 16, 16).T  # [16, seg/16]
        blocks.append(np.tile(a, (8, 1)))
    return np.concatenate(blocks, axis=1).copy()


# -------------------------------------------------------------- device build

def _build(nc, plan):
    skip_gather = bool(int(os.environ.get("GNN_SKIP_GATHER", "0")))
    skip_cc = bool(int(os.environ.get("GNN_SKIP_CC", "0")))
    seg_nch = plan["seg_nch"]
    seg_base = plan["seg_base"]
    span = plan["span"]
    nslots = plan["nslots"]
    nchunk = nslots // 128      # dstrel columns per core
    idx_cols = nslots // 16
    bat_chunks = int(max(seg_nch[b, 0] + seg_nch[b, 1] for b in range(NB)))
    ohw = 2 * plan["maxspan"]   # one-hot chunks per window (lo + hi spans)

    def din(name, shape, dt):
        return nc.dram_tensor(name, shape, dt, kind="ExternalInput").ap()

    xin = din("xin", [32, NPC], BF16)
    idx = din("idx", [128, idx_cols], I16)
    dstrel = din("dstrel", [128, nchunk], BF16)
    iota = din("iota", [128, 256], BF16)
    wn0 = din("wn0", [32, 128], BF16)
    wr0 = din("wr0", [32, 128], BF16)
    wn12 = din("wn12", [2, 128, 128], BF16)
    wr12 = din("wr12", [2, 128, 128], BF16)
    cb = din("cb", [128, 3], F32)
    bng = din("bng", [128, 3], F32)
    bnb = din("bnb", [128, 3], F32)
    gsw1 = din("gsw1", [128, 128], BF16)
    gsw2 = din("gsw2", [128, 128], BF16)
    ghw1 = din("ghw1", [128, 128], BF16)
    ghw2 = din("ghw2", [128, 64], BF16)
    ghw3 = din("ghw3", [64, 2], BF16)
    gsb1 = din("gsb1", [128, 1], F32)
    gsb2 = din("gsb2", [128, 1], F32)
    ghb1 = din("ghb1", [128, 1], F32)
    ghb2 = din("ghb2", [64, 1], F32)
    ghb3 = din("ghb3", [2, 1], F32)
    nhw1 = din("nhw1", [128, 128 * 128], BF16)
    nhw2 = din("nhw2", [128, 128 * 64], BF16)
    nhw3 = din("nhw3", [64, 128], BF16)
    nhb1 = din("nhb1", [128, 128], F32)
    nhb2 = din("nhb2", [64, 128], F32)
    nhb3 = din("nhb3", [1, 128], F32)

    outg = nc.dram_tensor("outg", [2, GPC], F32, kind="ExternalOutput").ap()
    outn = nc.dram_tensor("outn", [128, GPC], F32, kind="ExternalOutput").ap()

    y_local = nc.dram_tensor("y_local", [NPC, 128], BF16).ap()
    y_full = nc.dram_tensor("y_full", [N, 128], BF16, addr_space="Shared").ap()
    bn_in = [nc.dram_tensor(f"bn_in{l}", [128, 2], F32).ap() for l in range(3)]
    bn_out = [
        nc.dram_tensor(f"bn_out{l}", [128, 2], F32, addr_space="Shared").ap()
        for l in range(3)
    ]

    rg = [list(range(NC))]
    NQ = 4  # SWDGE queues; rotating queues spreads gather drains over rings

    with tile.TileContext(nc) as tc:
        with (
            tc.tile_pool(name="persist", bufs=1) as pp,
            tc.tile_pool(name="small", bufs=2) as sp,
        ):
            # --- persistent tiles / constants
            x_bf = pp.tile([32, NPC], BF16)
            nc.sync.dma_start(out=x_bf[:], in_=xin)
            h_bf = pp.tile([128, NPC], BF16)
            h_raw = pp.tile([128, NPC], F32)
            idx_sb = pp.tile([128, idx_cols], I16)
            nc.sync.dma_start(out=idx_sb[:], in_=idx)
            dr_sb = pp.tile([128, nchunk], BF16)
            nc.sync.dma_start(out=dr_sb[:], in_=dstrel)
            iota_sb = pp.tile([128, 256], BF16)
            nc.sync.dma_start(out=iota_sb[:], in_=iota)

            wn_sb = pp.tile([128, 3, 128], BF16)
            wr_sb = pp.tile([128, 3, 128], BF16)
            nc.sync.dma_start(out=wn_sb[:32, 0, :], in_=wn0)
            nc.sync.dma_start(out=wr_sb[:32, 0, :], in_=wr0)
            for l in range(2):
                nc.sync.dma_start(out=wn_sb[:, l + 1, :], in_=wn12[l])
                nc.sync.dma_start(out=wr_sb[:, l + 1, :], in_=wr12[l])
            cb_sb = pp.tile([128, 3], F32)
            nc.sync.dma_start(out=cb_sb[:], in_=cb)
            bng_sb = pp.tile([128, 3], F32)
            nc.sync.dma_start(out=bng_sb[:], in_=bng)
            bnb_sb = pp.tile([128, 3], F32)
            nc.sync.dma_start(out=bnb_sb[:], in_=bnb)

            # --- 3 GraphConv + BN + ReLU layers
            with (
                tc.tile_pool(name="lay", bufs=2) as lp,
                tc.tile_pool(name="ystage", bufs=1) as yp,
                tc.tile_pool(name="msg", bufs=4) as mp,
                tc.tile_pool(name="oh", bufs=4) as op_,
                tc.tile_pool(name="psA", bufs=2, space="PSUM") as psA,
                tc.tile_pool(name="psC", bufs=2, space="PSUM") as psC,
                tc.tile_pool(name="psW", bufs=3, space="PSUM") as psW,
            ):
                for l in range(3):
                    K = 32 if l == 0 else 128
                    hin = x_bf if l == 0 else h_bf
                    wn_l = wn_sb[:K, l, :]
                    wr_l = wr_sb[:K, l, :]
                    # per-window partial BN sums, filled as windows finish so
                    # the statistics overlap the gathers on the vector engine
                    s1w = lp.tile([128, NW], F32, tag="s1w")
                    s2w = lp.tile([128, NW], F32, tag="s2w")

                    # A+B) y_local = (h^T @ wn) node-major bf16, AllGathered
                    # in two halves so the first collective overlaps the
                    # second half's matmuls. y_full layout: [:HALF] holds
                    # every core's nodes 0:4096, [HALF:] nodes 4096:8192.
                    ystage = yp.tile([128, NW, 128], BF16, tag="ystage")
                    HB = NW // 2
                    for half in range(2):
                        for blk in range(half * HB, (half + 1) * HB):
                            yps = psA.tile(
                                [128, 128], F32, space="PSUM", tag="ypsum"
                            )
                            nc.tensor.matmul(
                                out=yps[:],
                                lhsT=hin[:, blk * 128 : (blk + 1) * 128],
                                rhs=wn_l,
                                start=True,
                                stop=True,
                            )
                            nc.vector.tensor_copy(
                                out=ystage[:, blk, :], in_=yps[:]
                            )
                        nc.sync.dma_start(
                            out=y_local[
                                half * (NPC // 2) : (half + 1) * (NPC // 2)
                            ].rearrange("(b p) f -> p b f", p=128),
                            in_=ystage[:, half * HB : (half + 1) * HB, :],
                        )
                        if not skip_cc:
                            nc.gpsimd.collective_compute(
                                "AllGather",
                                OP.bypass,
                                replica_groups=rg,
                                ins=[
                                    y_local[
                                        half * (NPC // 2) : (half + 1)
                                        * (NPC // 2)
                                    ].opt()
                                ],
                                outs=[
                                    y_full[
                                        half * HALF : (half + 1) * HALF
                                    ].opt()
                                ],
                            )

                    # C) root transform -> h_raw (bias folded into BN shift)
                    for t in range(16):
                        rps = psC.tile([128, 512], F32, space="PSUM", tag="cpsum")
                        nc.tensor.matmul(
                            out=rps[:],
                            lhsT=wr_l,
                            rhs=hin[:, t * 512 : (t + 1) * 512],
                            start=True,
                            stop=True,
                        )
                        nc.vector.tensor_copy(
                            out=h_raw[:, t * 512 : (t + 1) * 512], in_=rps[:]
                        )

                    # D) edge aggregation. The gather instruction is pure
                    # descriptor generation on gpsimd (~7.3ns/slot); the
                    # DMA drain runs asynchronously and overlaps the next
                    # call's generation. Queues rotate to spread drains.
                    for b in range(NB):
                        # single queue: each SDMA engine then drains one
                        # src-sorted descriptor stream at a time instead of
                        # round-robin interleaving 4 batches' streams, which
                        # destroys the HBM page locality the sort creates.
                        # Ring back-pressure on desc-gen no longer matters:
                        # generation is ~25us/layer vs ~240us of drain.
                        q = 0
                        nch_lo = int(seg_nch[b, 0])
                        nch_hi = int(seg_nch[b, 1])
                        icol_lo = int(seg_base[b, 0]) // 16
                        icol_hi = int(seg_base[b, 1]) // 16
                        msg = mp.tile([128, bat_chunks, 128], BF16, tag="msg")
                        if not skip_gather:
                            # single_packet must be False: with True, calls
                            # over 1024 indices crash and all descriptors
                            # drain through one SDMA engine (~8.7us/call).
                            nc.gpsimd.dma_gather(
                                msg[:, :nch_lo, :],
                                y_full[:HALF],
                                idx_sb[:, icol_lo : icol_lo + nch_lo * 8],
                                nch_lo * 128,
                                nch_lo * 128,
                                128,
                                single_packet=False,
                                queue_num=q,
                            )
                            nc.gpsimd.dma_gather(
                                msg[:, nch_lo : nch_lo + nch_hi, :],
                                y_full[HALF:],
                                idx_sb[:, icol_hi : icol_hi + nch_hi * 8],
                                nch_hi * 128,
                                nch_hi * 128,
                                128,
                                single_packet=False,
                                queue_num=q,
                            )
                        for wi in range(WPB):
                            w = b * WPB + wi
                            c0l, c1l = int(span[w, 0, 0]), int(span[w, 0, 1])
                            c0h, c1h = int(span[w, 1, 0]), int(span[w, 1, 1])
                            nl = c1l - c0l + 1
                            nh = c1h - c0h + 1
                            drl = int(seg_base[b, 0]) // 128 + c0l
                            drh = int(seg_base[b, 1]) // 128 + c0h
                            # even windows match dstrel 0..127, odd 128..255
                            it = iota_sb[:, 128 * (wi % 2) : 128 * (wi % 2) + 128]
                            oh = op_.tile([128, ohw, 128], BF16, tag="oh")
                            nc.vector.tensor_tensor(
                                out=oh[:, :nl, :],
                                in0=it.rearrange("p (c f) -> p c f", c=1)
                                .to_broadcast([128, nl, 128]),
                                in1=dr_sb[:, drl : drl + nl]
                                .rearrange("p (c f) -> p c f", f=1)
                                .to_broadcast([128, nl, 128]),
                                op=OP.is_equal,
                            )
                            nc.vector.tensor_tensor(
                                out=oh[:, nl : nl + nh, :],
                                in0=it.rearrange("p (c f) -> p c f", c=1)
                                .to_broadcast([128, nh, 128]),
                                in1=dr_sb[:, drh : drh + nh]
                                .rearrange("p (c f) -> p c f", f=1)
                                .to_broadcast([128, nh, 128]),
                                op=OP.is_equal,
                            )
                            wps = psW.tile([128, 128], F32, space="PSUM", tag="wpsum")
                            for j in range(nl + nh):
                                slot = (
                                    c0l + j
                                    if j < nl
                                    else nch_lo + c0h + (j - nl)
                                )
                                nc.tensor.matmul(
                                    out=wps[:],
                                    lhsT=msg[:, slot, :],
                                    rhs=oh[:, j, :],
                                    start=(j == 0),
                                    stop=(j == nl + nh - 1),
                                )
                            w = b * WPB + wi
                            nc.vector.tensor_tensor(
                                out=h_raw[:, w * 128 : (w + 1) * 128],
                                in0=h_raw[:, w * 128 : (w + 1) * 128],
                                in1=wps[:],
                                op=OP.add,
                            )
                            nc.vector.tensor_reduce(
                                out=s1w[:, w : w + 1],
                                in_=h_raw[:, w * 128 : (w + 1) * 128],
                                axis=mybir.AxisListType.X,
                                op=OP.add,
                            )
                            sqw = sp.tile([128, 128], F32, tag="sqw")
                            nc.vector.tensor_tensor(
                                out=sqw[:],
                                in0=h_raw[:, w * 128 : (w + 1) * 128],
                                in1=h_raw[:, w * 128 : (w + 1) * 128],
                                op=OP.mult,
                            )
                            nc.vector.tensor_reduce(
                                out=s2w[:, w : w + 1],
                                in_=sqw[:],
                                axis=mybir.AxisListType.X,
                                op=OP.add,
                            )

                    # E) BN statistics (sum, sumsq) + AllReduce
                    stats = sp.tile([128, 2], F32, tag="stats")
                    nc.vector.tensor_reduce(
                        out=stats[:, 0:1],
                        in_=s1w[:],
                        axis=mybir.AxisListType.X,
                        op=OP.add,
                    )
                    nc.vector.tensor_reduce(
                        out=stats[:, 1:2],
                        in_=s2w[:],
                        axis=mybir.AxisListType.X,
                        op=OP.add,
                    )
                    nc.sync.dma_start(out=bn_in[l], in_=stats[:])
                    nc.gpsimd.collective_compute(
                        "AllReduce",
                        OP.add,
                        replica_groups=rg,
                        ins=[bn_in[l].opt()],
                        outs=[bn_out[l].opt()],
                    )
                    gstats = sp.tile([128, 2], F32, tag="gstats")
                    nc.sync.dma_start(out=gstats[:], in_=bn_out[l])

                    # F) scale/shift: m = s1/N + cb; v = s2/N - (s1/N)^2
                    pr = sp.tile([128, 6], F32, tag="bnpar")
                    nc.vector.tensor_scalar_mul(pr[:, 0:1], gstats[:, 0:1], 1.0 / N)
                    nc.vector.tensor_scalar_mul(pr[:, 1:2], gstats[:, 1:2], 1.0 / N)
                    nc.vector.tensor_tensor(
                        out=pr[:, 2:3], in0=pr[:, 0:1], in1=pr[:, 0:1], op=OP.mult
                    )
                    nc.vector.tensor_tensor(
                        out=pr[:, 1:2], in0=pr[:, 1:2], in1=pr[:, 2:3],
                        op=OP.subtract,
                    )
                    nc.vector.tensor_scalar_add(pr[:, 1:2], pr[:, 1:2], EPS)
                    nc.scalar.sqrt(out=pr[:, 2:3], in_=pr[:, 1:2])
                    nc.vector.reciprocal(out=pr[:, 3:4], in_=pr[:, 2:3])
                    # scale = g * rstd
                    nc.vector.tensor_tensor(
                        out=pr[:, 3:4], in0=pr[:, 3:4],
                        in1=bng_sb[:, l : l + 1], op=OP.mult,
                    )
                    # m = s1/N + conv bias
                    nc.vector.tensor_tensor(
                        out=pr[:, 0:1], in0=pr[:, 0:1],
                        in1=cb_sb[:, l : l + 1], op=OP.add,
                    )
                    # shift = bnb - m * scale
                    nc.vector.tensor_tensor(
                        out=pr[:, 4:5], in0=pr[:, 0:1], in1=pr[:, 3:4], op=OP.mult
                    )
                    nc.vector.tensor_tensor(
                        out=pr[:, 5:6], in0=bnb_sb[:, l : l + 1],
                        in1=pr[:, 4:5], op=OP.subtract,
                    )

                    # G) h = relu(h_raw * scale + shift), bf16
                    for t in range(4):
                        nc.scalar.activation(
                            out=h_bf[:, t * 2048 : (t + 1) * 2048],
                            in_=h_raw[:, t * 2048 : (t + 1) * 2048],
                            func=AF.Relu,
                            bias=pr[:, 5:6],
                            scale=pr[:, 3:4],
                        )

            # --- heads (layers-scope pools are closed; SBUF freed)
            with (
                tc.tile_pool(name="hw", bufs=2) as hwp,
                tc.tile_pool(name="hsb", bufs=3) as hsb,
                tc.tile_pool(name="hps", bufs=2, space="PSUM") as hps,
                tc.tile_pool(name="hcst", bufs=1) as hc,
            ):
                # graph pooling (contiguous 128-node graphs) + graph head
                pool = hc.tile([128, GPC], F32)
                nc.vector.tensor_reduce(
                    out=pool[:],
                    in_=h_bf[:].rearrange("p (g n) -> p g n", g=GPC),
                    axis=mybir.AxisListType.X,
                    op=OP.add,
                )
                gw = {}
                for name, apw, shape in (
                    ("gsw1", gsw1, [128, 128]),
                    ("gsw2", gsw2, [128, 128]),
                    ("ghw1", ghw1, [128, 128]),
                    ("ghw2", ghw2, [128, 64]),
                    ("ghw3", ghw3, [64, 2]),
                ):
                    t = hc.tile(shape, BF16, tag=name)
                    nc.sync.dma_start(out=t[:], in_=apw)
                    gw[name] = t
                gb = {}
                for name, apb, p in (
                    ("gsb1", gsb1, 128),
                    ("gsb2", gsb2, 128),
                    ("ghb1", ghb1, 128),
                    ("ghb2", ghb2, 64),
                    ("ghb3", ghb3, 2),
                ):
                    t = hc.tile([p, 1], F32, tag=name)
                    nc.sync.dma_start(out=t[:], in_=apb)
                    gb[name] = t

                g0 = hsb.tile([128, GPC], BF16, tag="g0")
                nc.scalar.activation(
                    out=g0[:], in_=pool[:], func=AF.Relu, scale=1.0 / NPG
                )
                gp1 = hps.tile([128, GPC], F32, space="PSUM", tag="gps")
                nc.tensor.matmul(
                    out=gp1[:], lhsT=gw["gsw1"][:], rhs=g0[:], start=True, stop=True
                )
                g1 = hsb.tile([128, GPC], BF16, tag="g1")
                nc.vector.tensor_scalar_add(g1[:], gp1[:], gb["gsb1"][:])
                gp2 = hps.tile([128, GPC], F32, space="PSUM", tag="gps")
                nc.tensor.matmul(
                    out=gp2[:], lhsT=gw["gsw2"][:], rhs=g1[:], start=True, stop=True
                )
                g2 = hsb.tile([128, GPC], BF16, tag="g2")
                nc.scalar.activation(
                    out=g2[:], in_=gp2[:], func=AF.Relu, bias=gb["gsb2"][:]
                )
                gp3 = hps.tile([128, GPC], F32, space="PSUM", tag="gps")
                nc.tensor.matmul(
                    out=gp3[:], lhsT=gw["ghw1"][:], rhs=g2[:], start=True, stop=True
                )
                g3 = hsb.tile([128, GPC], BF16, tag="g3")
                nc.scalar.activation(
                    out=g3[:], in_=gp3[:], func=AF.Relu, bias=gb["ghb1"][:]
                )
                gp4 = hps.tile([64, GPC], F32, space="PSUM", tag="gps")
                nc.tensor.matmul(
                    out=gp4[:], lhsT=gw["ghw2"][:], rhs=g3[:], start=True, stop=True
                )
                g4 = hsb.tile([64, GPC], BF16, tag="g4")
                nc.scalar.activation(
                    out=g4[:], in_=gp4[:], func=AF.Relu, bias=gb["ghb2"][:]
                )
                gp5 = hps.tile([2, GPC], F32, space="PSUM", tag="gps")
                nc.tensor.matmul(
                    out=gp5[:], lhsT=gw["ghw3"][:], rhs=g4[:], start=True, stop=True
                )
                gout = hsb.tile([2, GPC], F32, tag="gout")
                nc.vector.tensor_scalar_add(gout[:], gp5[:], gb["ghb3"][:])
                nc.sync.dma_start(out=outg, in_=gout[:])

                # node heads: 128 positions x (128->128->64->1), 64 graphs each
                nb1 = hc.tile([128, 128], F32, tag="nb1")
                nc.sync.dma_start(out=nb1[:], in_=nhb1)
                nb2 = hc.tile([64, 128], F32, tag="nb2")
                nc.sync.dma_start(out=nb2[:], in_=nhb2)
                nb3 = hc.tile([1, 128], F32, tag="nb3")
                nc.sync.dma_start(out=nb3[:], in_=nhb3)
                w3 = hc.tile([64, 128], BF16, tag="w3")
                nc.sync.dma_start(out=w3[:], in_=nhw3)
                # single-partition accumulator: engines can't write at a
                # nonzero partition offset, so row p lives at cols [p*GPC,...)
                out_n = hc.tile([1, NPG * GPC], F32, tag="out_n")

                PCHUNK = 16
                for pc in range(NPG // PCHUNK):
                    w1 = hwp.tile([128, PCHUNK * 128], BF16, tag="w1")
                    nc.sync.dma_start(
                        out=w1[:],
                        in_=nhw1[:, pc * PCHUNK * 128 : (pc + 1) * PCHUNK * 128],
                    )
                    w2 = hwp.tile([128, PCHUNK * 64], BF16, tag="w2")
                    nc.sync.dma_start(
                        out=w2[:],
                        in_=nhw2[:, pc * PCHUNK * 64 : (pc + 1) * PCHUNK * 64],
                    )
                    for pi in range(PCHUNK):
                        p = pc * PCHUNK + pi
                        zp1 = hps.tile([128, GPC], F32, space="PSUM", tag="zp1")
                        nc.tensor.matmul(
                            out=zp1[:],
                            lhsT=w1[:, pi * 128 : (pi + 1) * 128],
                            rhs=h_bf[:, p :: NPG],
                            start=True,
                            stop=True,
                        )
                        z1 = hsb.tile([128, GPC], BF16, tag="z1")
                        nc.scalar.activation(
                            out=z1[:], in_=zp1[:], func=AF.Relu,
                            bias=nb1[:, p : p + 1],
                        )
                        zp2 = hps.tile([64, GPC], F32, space="PSUM", tag="zp2")
                        nc.tensor.matmul(
                            out=zp2[:],
                            lhsT=w2[:, pi * 64 : (pi + 1) * 64],
                            rhs=z1[:],
                            start=True,
                            stop=True,
                        )
                        z2 = hsb.tile([64, GPC], BF16, tag="z2")
                        nc.scalar.activation(
                            out=z2[:], in_=zp2[:], func=AF.Relu,
                            bias=nb2[:, p : p + 1],
                        )
                        zp3 = hps.tile([1, GPC], F32, space="PSUM", tag="zp3")
                        nc.tensor.matmul(
                            out=zp3[:],
                            lhsT=w3[:, p : p + 1],
                            rhs=z2[:],
                            start=True,
                            stop=True,
                        )
                        nc.vector.tensor_scalar_add(
                            out_n[:, p * GPC : (p + 1) * GPC],
                            zp3[:],
                            nb3[:, p : p + 1],
                        )
                nc.sync.dma_start(
                    out=outn.rearrange("(o p) g -> o (p g)", o=1), in_=out_n[:]
                )

    nc.compile()


# ------------------------------------------------------------------- driver

def _prep_inputs(inputs):
    f32 = lambda k: np.asarray(inputs[k], np.float32)
    bf16 = lambda a: np.ascontiguousarray(a).astype(NBF)

    edge_index = np.asarray(inputs["edge_index"], np.int64)
    plan, idx_all, dstrel_all = _build_edge_plan(edge_index)

    x = f32("x")
    iota = np.tile(np.arange(256, dtype=np.float32), (128, 1))

    shared = {
        "iota": bf16(iota),
        "wn0": bf16(f32("conv0_wn")),
        "wr0": bf16(f32("conv0_wr")),
        "wn12": bf16(f32("convs_wn")),
        "wr12": bf16(f32("convs_wr")),
        "cb": np.stack(
            [f32("conv0_b"), f32("convs_b")[0], f32("convs_b")[1]], axis=1
        ).copy(),
        "bng": np.stack(
            [f32("bn0_g"), f32("bns_g")[0], f32("bns_g")[1]], axis=1
        ).copy(),
        "bnb": np.stack(
            [f32("bn0_b"), f32("bns_b")[0], f32("bns_b")[1]], axis=1
        ).copy(),
        "gsw1": bf16(f32("gs_w1")),
        "gsw2": bf16(f32("gs_w2")),
        "ghw1": bf16(f32("gh_w1")),
        "ghw2": bf16(f32("gh_w2")),
        "ghw3": bf16(f32("gh_w3")),
        "gsb1": f32("gs_b1").reshape(128, 1).copy(),
        "gsb2": f32("gs_b2").reshape(128, 1).copy(),
        "ghb1": f32("gh_b1").reshape(128, 1).copy(),
        "ghb2": f32("gh_b2").reshape(64, 1).copy(),
        "ghb3": f32("gh_b3").reshape(2, 1).copy(),
        "nhw1": bf16(f32("nh_w1").transpose(1, 0, 2).reshape(128, 128 * 128)),
        "nhw2": bf16(f32("nh_w2").transpose(1, 0, 2).reshape(128, 128 * 64)),
        "nhw3": bf16(f32("nh_w3")[:, :, 0].T),
        "nhb1": f32("nh_b1").T.copy(),
        "nhb2": f32("nh_b2").T.copy(),
        "nhb3": f32("nh_b3").T.copy(),
    }

    seg_nch = plan["seg_nch"]
    seg_base = plan["seg_base"]
    in_maps = []
    for c in range(NC):
        # wrap per (batch, half) gather segment
        segs = []
        flat = idx_all[c]
        for b in range(NB):
            for h in range(2):
                base = int(seg_base[b, h])
                seglen = int(seg_nch[b, h]) * 128
                segs.append(_wrap_idx16(flat[base : base + seglen], seglen))
        idx_w = np.concatenate(segs, axis=1).copy()
        dr = dstrel_all[c].reshape(-1, 128).T.astype(NBF).copy()  # chunk-major
        in_maps.append(
            dict(
                shared,
                xin=bf16(x[c * NPC : (c + 1) * NPC].T),
                idx=idx_w,
                dstrel=dr,
            )
        )
    return plan, in_maps


def _numpy_fallback(inputs):
    """Reference math in numpy for unexpected input layouts."""
    f = lambda k: np.asarray(inputs[k], np.float32)
    x = f("x")
    src, dst = np.asarray(inputs["edge_index"], np.int64)
    batch = np.asarray(inputs["batch"], np.int64)

    def gconv(h, wr, wn, b):
        y = h @ wn
        agg = np.zeros_like(h @ wr)
        np.add.at(agg, dst, y[src])
        return h @ wr + agg + b

    def bn(h, g, bt):
        m = h.mean(0)
        v = h.var(0)
        return (h - m) / np.sqrt(v + EPS) * g + bt

    h = np.maximum(bn(gconv(x, f("conv0_wr"), f("conv0_wn"), f("conv0_b")),
                      f("bn0_g"), f("bn0_b")), 0)
    for i in range(2):
        h = np.maximum(
            bn(gconv(h, f("convs_wr")[i], f("convs_wn")[i], f("convs_b")[i]),
               f("bns_g")[i], f("bns_b")[i]), 0)
    counts = np.bincount(batch, minlength=B).astype(np.float32)
    xg = np.zeros((B, H), np.float32)
    np.add.at(xg, batch, h)
    xg /= counts[:, None]
    g = np.maximum(xg, 0)
    g = g @ f("gs_w1") + f("gs_b1")
    g = np.maximum(g @ f("gs_w2") + f("gs_b2"), 0)
    g = np.maximum(g @ f("gh_w1") + f("gh_b1"), 0)
    g = np.maximum(g @ f("gh_w2") + f("gh_b2"), 0)
    g = g @ f("gh_w3") + f("gh_b3")
    xn = h.reshape(B, NPG, H)
    z = np.maximum(np.einsum("bnf,nfh->bnh", xn, f("nh_w1")) + f("nh_b1"), 0)
    z = np.maximum(np.einsum("bnh,nhk->bnk", z, f("nh_w2")) + f("nh_b2"), 0)
    z = np.einsum("bnk,nko->bno", z, f("nh_w3")) + f("nh_b3")
    return np.concatenate([g, z[:, :, 0]], axis=1).astype(np.float32)


def _run(inputs, trace=False, trace_kwargs=None):
    batch = np.asarray(inputs["batch"], np.int64)
    if not (
        np.array_equal(batch, np.arange(N, dtype=np.int64) // NPG)
        and np.asarray(inputs["x"]).shape == (N, 32)
        and np.asarray(inputs["edge_index"]).shape == (2, E)
    ):
        return _numpy_fallback(inputs), None

    plan, in_maps = _prep_inputs(inputs)
    nc = bacc.Bacc(
        "TRN2",
        target_bir_lowering=False,
        debug=False,
        num_devices=NC,
        num_swdge_queues=4,
    )
    _build(nc, plan)
    r = run_bass_kernel_spmd(
        nc, in_maps, list(range(NC)), trace=trace, **(trace_kwargs or {})
    )
    out = np.zeros((B, 2 + NPG), np.float32)
    for c in range(NC):
        out[c * GPC : (c + 1) * GPC, 0:2] = r.results[c]["outg"].T
        out[c * GPC : (c + 1) * GPC, 2:] = r.results[c]["outn"].T
    return out, r


def kernel(**inputs):
    out, _ = _run(inputs)
    return out

